# revision 12
# baseline (speedup 1.0000x reference)
"""AttentionBlock kernel for 8 Trainium2 NeuronCores.

Computes: y = x + proj(attention(qkv(groupnorm(x)))) for x [8, 512, 64, 64].
Sharding: pure data-parallel — one batch item per core, weights replicated.

Per-core pipeline (one batch item, c=512 channels, N=4096 tokens):
  1. GroupNorm(32 groups): bn_stats/bn_aggr per channel -> group reduce via
     small fp32 matmuls with constant masks -> per-channel affine (A, B) ->
     single fused tensor_scalar pass producing bf16 normalized activations.
  2. QKV 1x1 convs as bf16 matmuls (weights pre-transposed host-side).
     q, k in [c, N] layout; v produced directly transposed ([N, c]) so the
     attention PV matmul needs no on-chip transpose. Biases are added inside
     the PSUM accumulation via K=1 rank-1 matmuls.
  3. Attention without max-subtraction (scores here are bounded ~|1.5|):
     S^T tiles [keys=128, queries=512] = k.T @ q, exp fused into the
     PSUM->SBUF eviction on ScalarE; softmax denominator accumulated on PE
     with an all-ones stationary operand (gives the broadcast sum directly);
     normalization deferred past the proj matmul (diagonal scaling commutes);
     residual (x + bp, precomputed host-side) added during proj eviction.
"""

import numpy as np
import ml_dtypes

P = 128
C = 512
CT = C // P  # 4 channel tiles
N = 4096
NT = N // P  # 32 token tiles of 128
NCH = N // 512  # 8 query chunks of 512
GROUPS = 32
EPS = 1e-5
B = 8
SCALE = 1.0 / np.sqrt(np.float32(C))

_CACHE = {}


def _patch_tile_drain():
    """walrus in this container rejects >2 semaphore waits on one
    instruction; TileContext's tail drain aggregates one wait per live
    proc. Spill the excess onto extra SP no-ops before the barrier."""
    import bass_rust
    import concourse.tile as tile
    from concourse.vector_clock import ScopedClock

    if getattr(tile.TileContext, "_drain_waitspill_patched", False):
        return

    def _drain_and_barrier(self, tick_clock, wait_clock):
        nc = self.nc
        drain_inst = nc.sync.drain()
        wait_clock.add_sem_waits(
            drain_inst.ins, ScopedClock({None: tick_clock.global_clock})
        )
        si = drain_inst.ins.sync_info
        if si is not None and len(si.on_wait) > 2:
            waits = list(si.on_wait)
            si.on_wait = waits[:2]
            for i in range(2, len(waits), 2):
                nop = nc.sync.nop(nofuse=True, hint=f"waitspill{i}")
                nop.ins.sync_info = bass_rust.SyncInfo(
                    on_wait=waits[i : i + 2], on_update=[]
                )
        nc.all_engine_barrier()
        popped = nc._tile_sem_poison_stack.pop()
        assert popped is self._sem_poison
        nc.clear_and_free_semaphores(list(self.sems.allocated().values()))
        nc.all_engine_barrier()

    tile.TileContext._drain_and_barrier = _drain_and_barrier
    tile.TileContext._drain_waitspill_patched = True


_MAX_WAITS = 1


def _spill_excess_waits(nc):
    """walrus in this container rejects instructions with >2 semaphore
    waits. Rewrite the serialized module: move excess waits of any
    instruction onto same-engine NoOps inserted right before it."""
    import json

    orig_to_json = nc.to_json_bytes

    def patched_to_json_bytes():
        m = json.loads(orig_to_json())
        ctr = 0
        for f in m["functions"]:
            for bb in f["blocks"]:
                insts = bb.get("instructions")
                if not insts:
                    continue
                new = []
                for ins in insts:
                    si = ins.get("sync_info")
                    ow = (si or {}).get("on_wait") or []
                    if len(ow) > _MAX_WAITS:
                        excess, keep = ow[:-_MAX_WAITS], ow[-_MAX_WAITS:]
                        si["on_wait"] = keep
                        for j in range(0, len(excess), _MAX_WAITS):
                            ctr += 1
                            nop = {
                                "engine": ins["engine"],
                                "ins": [],
                                "name": f"WSPILL-{ctr}",
                                "opcode": "NoOp",
                                "outs": [],
                                "sync_info": {
                                    "on_update": [],
                                    "on_wait": excess[j : j + _MAX_WAITS],
                                },
                                "text_hint": "waitspill",
                            }
                            if ins.get("debug") is not None:
                                nop["debug"] = ins["debug"]
                            new.append(nop)
                    new.append(ins)
                bb["instructions"] = new
        return json.dumps(m).encode()

    nc.to_json_bytes = patched_to_json_bytes


def build_nc():
    """Build the per-core Bass program (identical on all 8 cores)."""
    import concourse.bass as bass
    import concourse.tile as tile
    from concourse import mybir

    _patch_tile_drain()

    f32 = mybir.dt.float32
    bf16 = mybir.dt.bfloat16
    AF = mybir.ActivationFunctionType

    nc = bass.Bass(name="attnblk", trn_type="TRN2")

    x_d = nc.dram_tensor("x", [C, N], f32, kind="ExternalInput")
    xb_d = nc.dram_tensor("xb", [C, N], f32, kind="ExternalInput")
    wq_d = nc.dram_tensor("wqT", [C, C], bf16, kind="ExternalInput")
    wk_d = nc.dram_tensor("wkT", [C, C], bf16, kind="ExternalInput")
    wv_d = nc.dram_tensor("wvT", [C, C], bf16, kind="ExternalInput")
    wp_d = nc.dram_tensor("wpT", [C, C], bf16, kind="ExternalInput")
    bq_d = nc.dram_tensor("bq", [C], bf16, kind="ExternalInput")
    bk_d = nc.dram_tensor("bk", [C], bf16, kind="ExternalInput")
    bv_d = nc.dram_tensor("bv", [C], bf16, kind="ExternalInput")
    gns_d = nc.dram_tensor("gns", [C], f32, kind="ExternalInput")
    gnb_d = nc.dram_tensor("gnb", [C], f32, kind="ExternalInput")
    out_d = nc.dram_tensor("out", [C, N], f32, kind="ExternalOutput")

    # Constant group masks (NEFF-embedded).
    cidx = np.arange(C)
    gidx = np.arange(GROUPS)
    mask_avg_np = ((cidx[:, None] // 16) == gidx[None, :]).astype(np.float32) / 16.0
    mask_sel_np = ((cidx[None, :] // 16) == gidx[:, None]).astype(np.float32)
    mavg_d = nc.inline_tensor(mask_avg_np, name="mask_avg")  # [512, 32]
    msel_d = nc.inline_tensor(mask_sel_np, name="mask_sel")  # [32, 512]

    x_t = x_d[:].rearrange("(ci p) n -> p ci n", p=P)
    xb_t = xb_d[:].rearrange("(ci p) n -> p ci n", p=P)
    out_t = out_d[:].rearrange("(ci p) n -> p ci n", p=P)

    with tile.TileContext(nc) as tc:
        const = tc.alloc_tile_pool(name="const", bufs=1)
        pmm = tc.alloc_tile_pool(name="pmm", bufs=2, space="PSUM")

        # ---- constants / weights into SBUF ----
        wq_sb = const.tile([P, CT, C], bf16)
        nc.sync.dma_start(wq_sb[:], wq_d[:].rearrange("(ci p) o -> p ci o", p=P))
        wk_sb = const.tile([P, CT, C], bf16)
        nc.sync.dma_start(wk_sb[:], wk_d[:].rearrange("(ci p) o -> p ci o", p=P))
        wv_sb = const.tile([P, CT, C], bf16)
        nc.sync.dma_start(wv_sb[:], wv_d[:].rearrange("(ci p) o -> p ci o", p=P))
        wp_sb = const.tile([P, CT, C], bf16)
        nc.sync.dma_start(wp_sb[:], wp_d[:].rearrange("(ci p) o -> p ci o", p=P))
        mavg_sb = const.tile([P, CT, GROUPS], f32)
        nc.sync.dma_start(mavg_sb[:], mavg_d[:].rearrange("(ci p) g -> p ci g", p=P))
        msel_sb = const.tile([GROUPS, CT, P], f32)
        nc.sync.dma_start(msel_sb[:], msel_d[:].rearrange("g (ci p) -> g ci p", p=P))
        bq_sb = const.tile([1, C], bf16)
        nc.sync.dma_start(bq_sb[:], bq_d[None, :])
        bk_sb = const.tile([1, C], bf16)
        nc.sync.dma_start(bk_sb[:], bk_d[None, :])
        bv_sb = const.tile([1, C], bf16)
        nc.sync.dma_start(bv_sb[:], bv_d[None, :])
        gns_sb = const.tile([P, CT], f32)
        nc.sync.dma_start(gns_sb[:], gns_d[:].rearrange("(ci p) -> p ci", p=P))
        gnb_sb = const.tile([P, CT], f32)
        nc.sync.dma_start(gnb_sb[:], gnb_d[:].rearrange("(ci p) -> p ci", p=P))
        ones_k1 = const.tile([1, P], bf16)
        nc.vector.memset(ones_k1[:], 1.0)
        ones_row = const.tile([1, 512], bf16)
        nc.vector.memset(ones_row[:], 1.0)
        ones_bc = const.tile([P, P], bf16)
        nc.vector.memset(ones_bc[:], 1.0)
        eps_sb = const.tile([P, 1], f32)
        nc.vector.memset(eps_sb[:], EPS)

        # ---- GroupNorm ----
        # Phase-scoped pools go on the right SBUF side: their release order
        # (pgn, px after GN; pxn after QKV) must be LIFO per (space, side).
        pxn = tc.alloc_tile_pool(name="pxn", bufs=1, side="right")
        xn_sb = pxn.tile([P, CT, N], bf16)

        px = tc.alloc_tile_pool(name="px", bufs=1, side="right")
        pgn = tc.alloc_tile_pool(name="pgn", bufs=2, side="right")

        x_sb = px.tile([P, CT, N], f32)
        for ci in range(CT):
            nc.sync.dma_start(x_sb[:, ci, :], x_t[:, ci, :])

        mv = pgn.tile([P, CT, 2], f32, tag="mv", bufs=1)
        msq = pgn.tile([P, CT, 2], f32, tag="msq", bufs=1)
        for ci in range(CT):
            stats = pgn.tile([P, 8, 6], f32, tag="stats")
            for s in range(8):
                nc.vector.bn_stats(stats[:, s, :], x_sb[:, ci, s * 512 : (s + 1) * 512])
            nc.vector.bn_aggr(mv[:, ci, :], stats[:])
            # msq = [mean, E[x^2]] per channel
            nc.vector.tensor_copy(msq[:, ci, 0:1], mv[:, ci, 0:1])
            nc.vector.tensor_mul(msq[:, ci, 1:2], mv[:, ci, 0:1], mv[:, ci, 0:1])
            nc.vector.tensor_add(msq[:, ci, 1:2], msq[:, ci, 1:2], mv[:, ci, 1:2])

        ps_g = pmm.tile([GROUPS, 2], f32, tag="mm")
        for ci in range(CT):
            nc.tensor.matmul(
                ps_g[:],
                mavg_sb[:, ci, :],
                msq[:, ci, :],
                start=(ci == 0),
                stop=(ci == CT - 1),
            )
        gsb = pgn.tile([GROUPS, 2], f32, tag="gsb", bufs=1)
        nc.vector.tensor_copy(gsb[:], ps_g[:])
        # g2 = [mean_g, rstd_g]
        g2 = pgn.tile([GROUPS, 2], f32, tag="g2", bufs=1)
        nc.vector.tensor_copy(g2[:, 0:1], gsb[:, 0:1])
        var_t = pgn.tile([GROUPS, 1], f32, tag="var", bufs=1)
        nc.vector.tensor_mul(var_t[:], gsb[:, 0:1], gsb[:, 0:1])
        nc.vector.tensor_sub(var_t[:], gsb[:, 1:2], var_t[:])
        sq_t = pgn.tile([GROUPS, 1], f32, tag="sq", bufs=1)
        nc.scalar.activation(sq_t[:], var_t[:], AF.Sqrt, bias=eps_sb[:GROUPS, :])
        nc.vector.reciprocal(g2[:, 1:2], sq_t[:])

        ab = pgn.tile([P, CT, 2], f32, tag="ab", bufs=1)
        for ci in range(CT):
            ps_bc = pmm.tile([P, 2], f32, tag="mm")
            nc.tensor.matmul(ps_bc[:], msel_sb[:, ci, :], g2[:], start=True, stop=True)
            # A = rstd_g(c) * gn_scale[c];  B = gn_bias[c] - mean_g(c) * A
            nc.vector.tensor_mul(ab[:, ci, 0:1], ps_bc[:, 1:2], gns_sb[:, ci : ci + 1])
            tmpb = pgn.tile([P, 1], f32, tag="tmpb")
            nc.vector.tensor_mul(tmpb[:], ps_bc[:, 0:1], ab[:, ci, 0:1])
            nc.vector.tensor_sub(ab[:, ci, 1:2], gnb_sb[:, ci : ci + 1], tmpb[:])

        for ci in range(CT):
            nc.vector.tensor_scalar(
                xn_sb[:, ci, :],
                x_sb[:, ci, :],
                ab[:, ci, 0:1],
                ab[:, ci, 1:2],
                op0=mybir.AluOpType.mult,
                op1=mybir.AluOpType.add,
            )
        pgn.release()
        px.release()

        # ---- QKV projections ----
        pbig = tc.alloc_tile_pool(name="pbig", bufs=1)
        q_sb = pbig.tile([P, CT, N], bf16)
        k_sb = pbig.tile([P, CT, N], bf16)
        vT_sb = pbig.tile([P, NT, C], bf16)

        for w_t, b_row, dest in ((wq_sb, bq_sb, q_sb), (wk_sb, bk_sb, k_sb)):
            for oci in range(CT):
                for nch in range(NCH):
                    ps = pmm.tile([P, 512], f32, tag="mm")
                    for ici in range(CT):
                        nc.tensor.matmul(
                            ps[:],
                            w_t[:, ici, oci * P : (oci + 1) * P],
                            xn_sb[:, ici, nch * 512 : (nch + 1) * 512],
                            start=(ici == 0),
                            stop=False,
                        )
                    # bias: out[m, n] += b[oc_tile m] * 1 — rank-1 via K=1 matmul
                    nc.tensor.matmul(
                        ps[:],
                        b_row[:, oci * P : (oci + 1) * P],
                        ones_row[:],
                        start=False,
                        stop=True,
                    )
                    nc.scalar.copy(dest[:, oci, nch * 512 : (nch + 1) * 512], ps[:])

        for mt in range(NT):
            ps = pmm.tile([P, 512], f32, tag="mm")
            for ici in range(CT):
                nc.tensor.matmul(
                    ps[:],
                    xn_sb[:, ici, mt * P : (mt + 1) * P],
                    wv_sb[:, ici, :],
                    start=(ici == 0),
                    stop=False,
                )
            nc.tensor.matmul(ps[:], ones_k1[:], bv_sb[:], start=False, stop=True)
            nc.scalar.copy(vT_sb[:, mt, :], ps[:])
        pxn.release()

        # ---- attention + proj + residual ----
        pE = tc.alloc_tile_pool(name="pE", bufs=1)
        pO = tc.alloc_tile_pool(name="pO", bufs=2)
        prs = tc.alloc_tile_pool(name="prs", bufs=2)
        pxb = tc.alloc_tile_pool(name="pxb", bufs=3)
        pu = tc.alloc_tile_pool(name="pu", bufs=3)
        prs_ps = tc.alloc_tile_pool(name="prs_ps", bufs=2, space="PSUM")
        po_ps = tc.alloc_tile_pool(name="po_ps", bufs=2, space="PSUM")
        pp_ps = tc.alloc_tile_pool(name="pp_ps", bufs=2, space="PSUM")

        for nch in range(NCH):
            nsl = slice(nch * 512, (nch + 1) * 512)
            E_sb = pE.tile([P, NT, 512], bf16, tag="E")
            ps_rs = prs_ps.tile([P, 512], f32, tag="rs")
            for mt in range(NT):
                ps_s = pmm.tile([P, 512], f32, tag="mm")
                for ci in range(CT):
                    nc.tensor.matmul(
                        ps_s[:],
                        k_sb[:, ci, mt * P : (mt + 1) * P],
                        q_sb[:, ci, nsl],
                        start=(ci == 0),
                        stop=(ci == CT - 1),
                    )
                nc.scalar.activation(E_sb[:, mt, :], ps_s[:], AF.Exp, scale=float(SCALE))
                nc.tensor.matmul(
                    ps_rs[:],
                    ones_bc[:],
                    E_sb[:, mt, :],
                    start=(mt == 0),
                    stop=(mt == NT - 1),
                )
            rsinv = prs.tile([P, 512], f32, tag="rsinv")
            nc.vector.reciprocal(rsinv[:], ps_rs[:])

            O_sb = pO.tile([P, CT, 512], bf16, tag="O")
            for ci in range(CT):
                ps_o = po_ps.tile([P, 512], f32, tag="o")
                for mt in range(NT):
                    nc.tensor.matmul(
                        ps_o[:],
                        vT_sb[:, mt, ci * P : (ci + 1) * P],
                        E_sb[:, mt, :],
                        start=(mt == 0),
                        stop=(mt == NT - 1),
                    )
                nc.scalar.copy(O_sb[:, ci, :], ps_o[:])

            for oci in range(CT):
                ps_p = pp_ps.tile([P, 512], f32, tag="p")
                for ici in range(CT):
                    nc.tensor.matmul(
                        ps_p[:],
                        wp_sb[:, ici, oci * P : (oci + 1) * P],
                        O_sb[:, ici, :],
                        start=(ici == 0),
                        stop=(ici == CT - 1),
                    )
                xb_tile = pxb.tile([P, 512], f32, tag="xb")
                nc.sync.dma_start(xb_tile[:], xb_t[:, oci, nsl])
                u = pu.tile([P, 512], f32, tag="u")
                nc.vector.tensor_mul(u[:], ps_p[:], rsinv[:])
                nc.vector.tensor_add(u[:], u[:], xb_tile[:])
                nc.sync.dma_start(out_t[:, oci, nsl], u[:])

        # LIFO release per (space, side) stack
        pu.release()
        pxb.release()
        prs.release()
        pO.release()
        pE.release()
        pbig.release()
        const.release()
        pp_ps.release()
        po_ps.release()
        prs_ps.release()
        pmm.release()

    _spill_excess_waits(nc)
    return nc


def _prep_inputs(x, gn_scale, gn_bias, wq, bq, wk, bk, wv, bv, wp, bp):
    bf = ml_dtypes.bfloat16
    x = np.asarray(x, dtype=np.float32).reshape(B, C, N)
    bp = np.asarray(bp, dtype=np.float32)
    xb = x + bp[None, :, None]
    shared = {
        "wqT": np.ascontiguousarray(np.asarray(wq, np.float32).T).astype(bf),
        "wkT": np.ascontiguousarray(np.asarray(wk, np.float32).T).astype(bf),
        "wvT": np.ascontiguousarray(np.asarray(wv, np.float32).T).astype(bf),
        "wpT": np.ascontiguousarray(np.asarray(wp, np.float32).T).astype(bf),
        "bq": np.asarray(bq, np.float32).astype(bf),
        "bk": np.asarray(bk, np.float32).astype(bf),
        "bv": np.asarray(bv, np.float32).astype(bf),
        "gns": np.asarray(gn_scale, np.float32),
        "gnb": np.asarray(gn_bias, np.float32),
    }
    in_maps = []
    for i in range(B):
        m = dict(shared)
        m["x"] = np.ascontiguousarray(x[i])
        m["xb"] = np.ascontiguousarray(xb[i])
        in_maps.append(m)
    return in_maps


def kernel(**inputs):
    from concourse.bass_utils import run_bass_kernel_spmd

    if "nc" not in _CACHE:
        _CACHE["nc"] = build_nc()
    nc = _CACHE["nc"]
    in_maps = _prep_inputs(**inputs)
    res = run_bass_kernel_spmd(nc, in_maps, core_ids=list(range(B)))
    _CACHE["last_exec_time_ns"] = res.exec_time_ns
    out = np.stack([np.asarray(r["out"]) for r in res.results])
    return out.reshape(B, C, 64, 64).astype(np.float32)


def last_exec_time_ns():
    return _CACHE.get("last_exec_time_ns")


# revision 37
# speedup vs baseline: 13333.0727x; 13333.0727x over previous
"""AttentionBlock kernel for 8 Trainium2 NeuronCores.

Computes: y = x + proj(attention(qkv(groupnorm(x)))) for x [8, 512, 64, 64].
Sharding: pure data-parallel — one batch item per core, weights replicated.

Per-core pipeline (one batch item, c=512 channels, N=4096 tokens):
  1. GroupNorm(32 groups): per 128-channel tile (groups never span tiles):
     bn_stats/bn_aggr per channel -> per-tile group reduce via small fp32
     matmuls with constant masks -> per-channel affine (A, B) -> single
     fused tensor_scalar pass producing bf16 normalized activations.
  2. QKV 1x1 convs as bf16 matmuls (weights pre-transposed host-side).
     q, k in [c, N] layout; v produced directly transposed ([N, c]) so the
     attention PV matmul needs no on-chip transpose. bq is added inside the
     PSUM accumulation via a K=1 rank-1 matmul. bk is dropped (it cancels
     exactly in softmax); bv commutes through attention (rows sum to 1) and
     is folded into the host-precomputed residual xb = x + bp + Wp @ bv.
  3. Attention without max-subtraction (scores here are bounded ~|1.5|):
     S^T tiles [keys=128, queries=512] = k.T @ q, exp fused into the
     PSUM->SBUF eviction on ScalarE; softmax denominator accumulated on
     VectorE (per-partition partials) + one all-ones matmul per chunk for
     the cross-partition broadcast sum; normalization deferred past the
     proj matmul (diagonal scaling commutes); residual added during proj
     eviction.
"""

import numpy as np
import ml_dtypes

P = 128
C = 512
CT = C // P  # 4 channel tiles
N = 4096
NT = N // P  # 32 token tiles of 128
NCH = N // 512  # 8 query chunks of 512
GPT = 8  # groups per 128-channel tile (group = 16 channels)
EPS = 1e-5
B = 8
SCALE = 1.0 / np.sqrt(np.float32(C))

_CACHE = {}
_MAX_WAITS = 1


def _patch_tile_drain():
    """walrus in this container rejects >1 semaphore wait on one
    instruction; TileContext's tail drain aggregates one wait per live
    proc. Spill the excess onto extra SP no-ops before the barrier."""
    import bass_rust
    import concourse.tile as tile
    from concourse.vector_clock import ScopedClock

    if getattr(tile.TileContext, "_drain_waitspill_patched", False):
        return

    def _drain_and_barrier(self, tick_clock, wait_clock):
        nc = self.nc
        drain_inst = nc.sync.drain()
        wait_clock.add_sem_waits(
            drain_inst.ins, ScopedClock({None: tick_clock.global_clock})
        )
        si = drain_inst.ins.sync_info
        if si is not None and len(si.on_wait) > _MAX_WAITS:
            waits = list(si.on_wait)
            si.on_wait = waits[:_MAX_WAITS]
            for i in range(_MAX_WAITS, len(waits), _MAX_WAITS):
                nop = nc.sync.nop(nofuse=True, hint=f"waitspill{i}")
                nop.ins.sync_info = bass_rust.SyncInfo(
                    on_wait=waits[i : i + _MAX_WAITS], on_update=[]
                )
        nc.all_engine_barrier()
        popped = nc._tile_sem_poison_stack.pop()
        assert popped is self._sem_poison
        nc.clear_and_free_semaphores(list(self.sems.allocated().values()))
        nc.all_engine_barrier()

    tile.TileContext._drain_and_barrier = _drain_and_barrier
    tile.TileContext._drain_waitspill_patched = True


def _spill_excess_waits(nc):
    """Rewrite the serialized module: move excess semaphore waits of any
    instruction onto same-engine NoOps inserted right before it (walrus
    here rejects instructions with more than one wait)."""
    import json

    orig_to_json = nc.to_json_bytes

    def patched_to_json_bytes():
        m = json.loads(orig_to_json())
        ctr = 0
        for f in m["functions"]:
            for bb in f["blocks"]:
                insts = bb.get("instructions")
                if not insts:
                    continue
                new = []
                for ins in insts:
                    si = ins.get("sync_info")
                    ow = (si or {}).get("on_wait") or []
                    if len(ow) > _MAX_WAITS:
                        excess, keep = ow[:-_MAX_WAITS], ow[-_MAX_WAITS:]
                        si["on_wait"] = keep
                        for j in range(0, len(excess), _MAX_WAITS):
                            ctr += 1
                            nop = {
                                "engine": ins["engine"],
                                "ins": [],
                                "name": f"WSPILL-{ctr}",
                                "opcode": "NoOp",
                                "outs": [],
                                "sync_info": {
                                    "on_update": [],
                                    "on_wait": excess[j : j + _MAX_WAITS],
                                },
                                "text_hint": "waitspill",
                            }
                            if ins.get("debug") is not None:
                                nop["debug"] = ins["debug"]
                            new.append(nop)
                    new.append(ins)
                bb["instructions"] = new
        return json.dumps(m).encode()

    nc.to_json_bytes = patched_to_json_bytes


def build_nc(e_bufs=2, fp8=False, fp8_full=False):
    """Build the per-core Bass program (identical on all 8 cores).

    fp8: q/k/vT/E in fp8e4, QK + PV + rowsum matmuls in DoubleRow.
    fp8_full: additionally xn/weights/O in fp8e4 so QKV + proj matmuls
    are DoubleRow too, and q/k/vT evictions move to VectorE (ScalarE is
    then dominated by the exp evictions).
    """
    import concourse.bass as bass
    import concourse.tile as tile
    from concourse import mybir

    if fp8_full:
        fp8 = True
    _patch_tile_drain()

    f32 = mybir.dt.float32
    bf16 = mybir.dt.bfloat16
    fp8e4 = mybir.dt.float8e4
    adt = fp8e4 if fp8 else bf16  # attention operand dtype (q, k, vT, E)
    wdt = fp8e4 if fp8_full else bf16  # qkv/proj weight + xn + O dtype
    DR = mybir.MatmulPerfMode.DoubleRow if fp8 else None
    AF = mybir.ActivationFunctionType

    nc = bass.Bass(name="attnblk", trn_type="TRN2")

    x_d = nc.dram_tensor("x", [C, N], f32, kind="ExternalInput")
    xb_d = nc.dram_tensor("xb", [C, N], f32, kind="ExternalInput")
    wq_d = nc.dram_tensor("wqT", [C, C], wdt, kind="ExternalInput")
    wk_d = nc.dram_tensor("wkT", [C, C], wdt, kind="ExternalInput")
    wv_d = nc.dram_tensor("wvT", [C, C], wdt, kind="ExternalInput")
    wp_d = nc.dram_tensor("wpT", [C, C], wdt, kind="ExternalInput")
    bq_d = nc.dram_tensor("bq", [C], bf16, kind="ExternalInput")
    gns_d = nc.dram_tensor("gns", [C], f32, kind="ExternalInput")
    gnb_d = nc.dram_tensor("gnb", [C], f32, kind="ExternalInput")
    out_d = nc.dram_tensor("out", [C, N], f32, kind="ExternalOutput")

    # Constant group masks (NEFF-embedded), per 128-channel tile:
    # pavg [128, 8]: p -> group p//16, value 1/16 (group average)
    # psel [8, 128]: one-hot selector transposed (group value -> channels)
    pidx = np.arange(P)
    gidx = np.arange(GPT)
    pavg_np = ((pidx[:, None] // 16) == gidx[None, :]).astype(np.float32) / 16.0
    psel_np = ((pidx[None, :] // 16) == gidx[:, None]).astype(np.float32)
    pavg_d = nc.inline_tensor(pavg_np, name="mask_avg")  # [128, 8]
    psel_d = nc.inline_tensor(psel_np, name="mask_sel")  # [8, 128]

    x_t = x_d[:].rearrange("(ci p) n -> p ci n", p=P)
    xb_t = xb_d[:].rearrange("(ci p) n -> p ci n", p=P)
    out_t = out_d[:].rearrange("(ci p) n -> p ci n", p=P)

    with tile.TileContext(nc) as tc:
        const = tc.alloc_tile_pool(name="const", bufs=1)
        pmm = tc.alloc_tile_pool(name="pmm", bufs=2, space="PSUM")

        # ---- constants / weights into SBUF ----
        wp_sb = const.tile([P, CT, C], wdt)
        nc.sync.dma_start(wp_sb[:], wp_d[:].rearrange("(ci p) o -> p ci o", p=P))
        pavg_sb = const.tile([P, GPT], f32)
        nc.sync.dma_start(pavg_sb[:], pavg_d[:])
        psel_sb = const.tile([GPT, P], f32)
        nc.sync.dma_start(psel_sb[:], psel_d[:])
        bq_sb = const.tile([1, C], bf16)
        nc.sync.dma_start(bq_sb[:], bq_d[None, :])
        gns_sb = const.tile([P, CT], f32)
        nc.sync.dma_start(gns_sb[:], gns_d[:].rearrange("(ci p) -> p ci", p=P))
        gnb_sb = const.tile([P, CT], f32)
        nc.sync.dma_start(gnb_sb[:], gnb_d[:].rearrange("(ci p) -> p ci", p=P))
        ones_row = const.tile([1, 512], bf16)
        nc.vector.memset(ones_row[:], 1.0)
        ones_bc = const.tile([P, P], f32)
        nc.vector.memset(ones_bc[:], 1.0)
        eps_sb = const.tile([P, 1], f32)
        nc.vector.memset(eps_sb[:], EPS)
        # Unnormalized O can exceed fp8e4's ±448 range; store O/o_scale in
        # fp8 and fold o_scale back in via the rowsum broadcast matmul.
        o_scale = 64.0 if fp8_full else 1.0
        if fp8:
            # all-ones for DoubleRow rowsum over key tiles; padded so the
            # k-interleave AP step is 16 bytes (DoubleRow requires step%16==0)
            ones2_t = const.tile([P, 2, 16], fp8e4)
            nc.vector.memset(ones2_t[:], 1.0)
            ones2 = ones2_t[:, :, 0:1]
            # [1, 128] constant for the K=1 rowsum broadcast matmul:
            # yields o_scale/rowsum broadcast across partitions
            ones_k1 = const.tile([1, P], f32)
            nc.vector.memset(ones_k1[:], o_scale)

        # QKV weights in a releasable pool (right side, LIFO with GN pools)
        pw = tc.alloc_tile_pool(name="pw", bufs=1, side="right")
        wq_sb = pw.tile([P, CT, C], wdt)
        nc.sync.dma_start(wq_sb[:], wq_d[:].rearrange("(ci p) o -> p ci o", p=P))
        wk_sb = pw.tile([P, CT, C], wdt)
        nc.sync.dma_start(wk_sb[:], wk_d[:].rearrange("(ci p) o -> p ci o", p=P))
        wv_sb = pw.tile([P, CT, C], wdt)
        nc.sync.dma_start(wv_sb[:], wv_d[:].rearrange("(ci p) o -> p ci o", p=P))

        # ---- GroupNorm (fully per-channel-tile: groups are 16 channels) ----
        pxn = tc.alloc_tile_pool(name="pxn", bufs=1, side="right")
        xn_sb = pxn.tile([P, CT, N], wdt)

        px = tc.alloc_tile_pool(name="px", bufs=1, side="right")
        pgn = tc.alloc_tile_pool(name="pgn", bufs=2, side="right")

        x_sb = px.tile([P, CT, N], f32)
        for ci in range(CT):
            nc.sync.dma_start(x_sb[:, ci, :], x_t[:, ci, :])

        for ci in range(CT):
            stats = pgn.tile([P, 8, 6], f32, tag="stats")
            for s in range(8):
                nc.vector.bn_stats(stats[:, s, :], x_sb[:, ci, s * 512 : (s + 1) * 512])
            mv = pgn.tile([P, 2], f32, tag="mv")
            nc.vector.bn_aggr(mv[:], stats[:])
            # msq = [mean, E[x^2]] per channel
            msq = pgn.tile([P, 2], f32, tag="msq")
            nc.vector.tensor_copy(msq[:, 0:1], mv[:, 0:1])
            nc.vector.tensor_mul(msq[:, 1:2], mv[:, 0:1], mv[:, 0:1])
            nc.vector.tensor_add(msq[:, 1:2], msq[:, 1:2], mv[:, 1:2])

            # group-average via mask matmul: [8 groups, 2]
            ps_g = pmm.tile([GPT, 2], f32, tag="mm")
            nc.tensor.matmul(ps_g[:], pavg_sb[:], msq[:], start=True, stop=True)
            g2 = pgn.tile([GPT, 2], f32, tag="g2")  # -> [mean_g, rstd_g]
            nc.vector.tensor_copy(g2[:, 0:1], ps_g[:, 0:1])
            var_t = pgn.tile([GPT, 1], f32, tag="var")
            nc.vector.tensor_mul(var_t[:], g2[:, 0:1], g2[:, 0:1])
            nc.vector.tensor_sub(var_t[:], ps_g[:, 1:2], var_t[:])
            sq_t = pgn.tile([GPT, 1], f32, tag="sq")
            nc.scalar.activation(sq_t[:], var_t[:], AF.Sqrt, bias=eps_sb[:GPT, :])
            nc.vector.reciprocal(g2[:, 1:2], sq_t[:])

            # broadcast group values back to channels: [128, 2]
            ps_bc = pmm.tile([P, 2], f32, tag="mm")
            nc.tensor.matmul(ps_bc[:], psel_sb[:], g2[:], start=True, stop=True)
            # A = rstd_g(c) * gn_scale[c];  B = gn_bias[c] - mean_g(c) * A
            ab = pgn.tile([P, 2], f32, tag="ab")
            nc.vector.tensor_mul(ab[:, 0:1], ps_bc[:, 1:2], gns_sb[:, ci : ci + 1])
            tmpb = pgn.tile([P, 1], f32, tag="tmpb")
            nc.vector.tensor_mul(tmpb[:], ps_bc[:, 0:1], ab[:, 0:1])
            nc.vector.tensor_sub(ab[:, 1:2], gnb_sb[:, ci : ci + 1], tmpb[:])

            nc.vector.tensor_scalar(
                xn_sb[:, ci, :],
                x_sb[:, ci, :],
                ab[:, 0:1],
                ab[:, 1:2],
                op0=mybir.AluOpType.mult,
                op1=mybir.AluOpType.add,
            )
        pgn.release()
        px.release()

        # ---- QKV projections ----
        pbig = tc.alloc_tile_pool(name="pbig", bufs=1)
        q_sb = pbig.tile([P, CT, N], adt)
        k_sb = pbig.tile([P, CT, N], adt)
        vT_sb = pbig.tile([P, NT, C], adt)

        def qkv_evict(dst, src):
            if fp8_full:
                nc.vector.tensor_copy(dst, src)
            else:
                nc.scalar.copy(dst, src)

        def proj_mms(ps, w_t, oci, rhs_sb, rhs_sl, last_stop):
            """ps += w_t[:, :, oci-tile].T @ rhs over the 4 ici tiles."""
            if fp8_full:
                for ici2 in range(0, CT, 2):
                    nc.tensor.matmul(
                        ps[:],
                        w_t[:, ici2 : ici2 + 2, oci * P : (oci + 1) * P],
                        rhs_sb[:, ici2 : ici2 + 2, rhs_sl],
                        start=(ici2 == 0),
                        stop=(ici2 == CT - 2) and last_stop,
                        perf_mode=DR,
                    )
            else:
                for ici in range(CT):
                    nc.tensor.matmul(
                        ps[:],
                        w_t[:, ici, oci * P : (oci + 1) * P],
                        rhs_sb[:, ici, rhs_sl],
                        start=(ici == 0),
                        stop=(ici == CT - 1) and last_stop,
                    )

        for oci in range(CT):
            for nch in range(NCH):
                nsl = slice(nch * 512, (nch + 1) * 512)
                ps = pmm.tile([P, 512], f32, tag="mm")
                proj_mms(ps, wq_sb, oci, xn_sb, nsl, last_stop=False)
                # bias: out[m, n] += bq[oci*128+m] * 1 — rank-1 K=1 matmul
                nc.tensor.matmul(
                    ps[:],
                    bq_sb[:, oci * P : (oci + 1) * P],
                    ones_row[:],
                    start=False,
                    stop=True,
                )
                qkv_evict(q_sb[:, oci, nsl], ps[:])

        for oci in range(CT):
            for nch in range(NCH):
                nsl = slice(nch * 512, (nch + 1) * 512)
                ps = pmm.tile([P, 512], f32, tag="mm")
                proj_mms(ps, wk_sb, oci, xn_sb, nsl, last_stop=True)
                qkv_evict(k_sb[:, oci, nsl], ps[:])

        for mt in range(NT):
            ps = pmm.tile([P, 512], f32, tag="mm")
            if fp8_full:
                for ici2 in range(0, CT, 2):
                    nc.tensor.matmul(
                        ps[:],
                        xn_sb[:, ici2 : ici2 + 2, mt * P : (mt + 1) * P],
                        wv_sb[:, ici2 : ici2 + 2, :],
                        start=(ici2 == 0),
                        stop=(ici2 == CT - 2),
                        perf_mode=DR,
                    )
            else:
                for ici in range(CT):
                    nc.tensor.matmul(
                        ps[:],
                        xn_sb[:, ici, mt * P : (mt + 1) * P],
                        wv_sb[:, ici, :],
                        start=(ici == 0),
                        stop=(ici == CT - 1),
                    )
            qkv_evict(vT_sb[:, mt, :], ps[:])
        pxn.release()
        pw.release()

        # ---- attention + proj + residual ----
        pE = tc.alloc_tile_pool(name="pE", bufs=e_bufs)
        pO = tc.alloc_tile_pool(name="pO", bufs=2)
        prs = tc.alloc_tile_pool(name="prs", bufs=2)
        pacc = tc.alloc_tile_pool(name="pacc", bufs=2)
        pxb = tc.alloc_tile_pool(name="pxb", bufs=3)
        pu = tc.alloc_tile_pool(name="pu", bufs=3)
        prs_ps = tc.alloc_tile_pool(name="prs_ps", bufs=2, space="PSUM")
        po_ps = tc.alloc_tile_pool(name="po_ps", bufs=2, space="PSUM")
        pp_ps = tc.alloc_tile_pool(name="pp_ps", bufs=2, space="PSUM")

        for nch in range(NCH):
            nsl = slice(nch * 512, (nch + 1) * 512)
            E_sb = pE.tile([P, NT, 512], adt, tag="E")
            if fp8:
                ps_rs1 = prs_ps.tile([1, 512], f32, tag="rsbc")
            else:
                racc = pacc.tile([P, 512], f32, tag="racc")
            for mt in range(NT):
                ps_s = pmm.tile([P, 512], f32, tag="mm")
                if fp8:
                    for ci2 in range(0, CT, 2):
                        nc.tensor.matmul(
                            ps_s[:],
                            k_sb[:, ci2 : ci2 + 2, mt * P : (mt + 1) * P],
                            q_sb[:, ci2 : ci2 + 2, nsl],
                            start=(ci2 == 0),
                            stop=(ci2 == CT - 2),
                            perf_mode=DR,
                        )
                else:
                    for ci in range(CT):
                        nc.tensor.matmul(
                            ps_s[:],
                            k_sb[:, ci, mt * P : (mt + 1) * P],
                            q_sb[:, ci, nsl],
                            start=(ci == 0),
                            stop=(ci == CT - 1),
                        )
                nc.scalar.activation(E_sb[:, mt, :], ps_s[:], AF.Exp, scale=float(SCALE))
                if fp8:
                    # rowsum on PE: DoubleRow all-ones contraction per m-pair
                    if mt % 2 == 1:
                        nc.tensor.matmul(
                            ps_rs1[:],
                            ones2[:],
                            E_sb[:, mt - 1 : mt + 1, :],
                            start=(mt == 1),
                            stop=(mt == NT - 1),
                            perf_mode=DR,
                        )
                else:
                    # rowsum partials on DVE: racc accumulates E over m-tiles
                    if mt == 0:
                        nc.vector.tensor_copy(racc[:], E_sb[:, 0, :])
                    else:
                        nc.vector.tensor_add(racc[:], racc[:], E_sb[:, mt, :])
            rsinv = prs.tile([P, 512], f32, tag="rsinv")
            if fp8:
                # 1/rowsum on one partition, then broadcast via K=1 matmul
                rs1 = prs.tile([1, 512], f32, tag="rs1")
                nc.vector.reciprocal(rs1[:], ps_rs1[:])
                ps_bc = prs_ps.tile([P, 512], f32, tag="rsbc")
                nc.tensor.matmul(ps_bc[:], ones_k1[:], rs1[:], start=True, stop=True)
                nc.vector.tensor_copy(rsinv[:], ps_bc[:])
            else:
                # cross-partition broadcast sum via all-ones matmul, then 1/x
                ps_rs = prs_ps.tile([P, 512], f32, tag="rs")
                nc.tensor.matmul(ps_rs[:], ones_bc[:], racc[:], start=True, stop=True)
                nc.vector.reciprocal(rsinv[:], ps_rs[:])

            O_sb = pO.tile([P, CT, 512], wdt, tag="O")
            for ci in range(CT):
                ps_o = po_ps.tile([P, 512], f32, tag="o")
                if fp8:
                    for mt2 in range(0, NT, 2):
                        nc.tensor.matmul(
                            ps_o[:],
                            vT_sb[:, mt2 : mt2 + 2, ci * P : (ci + 1) * P],
                            E_sb[:, mt2 : mt2 + 2, :],
                            start=(mt2 == 0),
                            stop=(mt2 == NT - 2),
                            perf_mode=DR,
                        )
                else:
                    for mt in range(NT):
                        nc.tensor.matmul(
                            ps_o[:],
                            vT_sb[:, mt, ci * P : (ci + 1) * P],
                            E_sb[:, mt, :],
                            start=(mt == 0),
                            stop=(mt == NT - 1),
                        )
                if fp8_full:
                    # keep ScalarE free; scale into fp8e4 range
                    nc.vector.tensor_scalar_mul(
                        O_sb[:, ci, :], ps_o[:], 1.0 / o_scale
                    )
                elif fp8:
                    # keep ScalarE free for the exp evictions
                    nc.vector.tensor_copy(O_sb[:, ci, :], ps_o[:])
                else:
                    nc.scalar.copy(O_sb[:, ci, :], ps_o[:])

            for oci in range(CT):
                ps_p = pp_ps.tile([P, 512], f32, tag="p")
                proj_mms(ps_p, wp_sb, oci, O_sb, slice(0, 512), last_stop=True)
                xb_tile = pxb.tile([P, 512], f32, tag="xb")
                nc.sync.dma_start(xb_tile[:], xb_t[:, oci, nsl])
                u = pu.tile([P, 512], f32, tag="u")
                nc.vector.tensor_mul(u[:], ps_p[:], rsinv[:])
                nc.vector.tensor_add(u[:], u[:], xb_tile[:])
                nc.sync.dma_start(out_t[:, oci, nsl], u[:])

        # LIFO release per (space, side) stack
        pu.release()
        pxb.release()
        pacc.release()
        prs.release()
        pO.release()
        pE.release()
        pbig.release()
        const.release()
        pp_ps.release()
        po_ps.release()
        prs_ps.release()
        pmm.release()

    _spill_excess_waits(nc)
    return nc


def _prep_inputs(
    x, gn_scale, gn_bias, wq, bq, wk, bk, wv, bv, wp, bp, fp8_full=False
):
    bf = ml_dtypes.bfloat16
    wdt = ml_dtypes.float8_e4m3 if fp8_full else bf
    x = np.asarray(x, dtype=np.float32).reshape(B, C, N)
    bp = np.asarray(bp, dtype=np.float32)
    bv = np.asarray(bv, dtype=np.float32)
    wp_f = np.asarray(wp, np.float32)
    # bv commutes through attention (attn rows sum to 1); residual gets
    # x + bp + Wp @ bv. bk cancels exactly in softmax and is dropped.
    resid_bias = bp + wp_f @ bv
    xb = x + resid_bias[None, :, None]
    shared = {
        "wqT": np.ascontiguousarray(np.asarray(wq, np.float32).T).astype(wdt),
        "wkT": np.ascontiguousarray(np.asarray(wk, np.float32).T).astype(wdt),
        "wvT": np.ascontiguousarray(np.asarray(wv, np.float32).T).astype(wdt),
        "wpT": np.ascontiguousarray(wp_f.T).astype(wdt),
        "bq": np.asarray(bq, np.float32).astype(bf),
        "gns": np.asarray(gn_scale, np.float32),
        "gnb": np.asarray(gn_bias, np.float32),
    }
    in_maps = []
    for i in range(B):
        m = dict(shared)
        m["x"] = np.ascontiguousarray(x[i])
        m["xb"] = np.ascontiguousarray(xb[i])
        in_maps.append(m)
    return in_maps


VARIANTS = {
    "bf16": dict(fp8=False, fp8_full=False),
    "fp8": dict(fp8=True, fp8_full=False),
    "fp8full": dict(fp8=True, fp8_full=True),
}


def _run_variant(variant, inputs, trace=False):
    from concourse.bass_utils import run_bass_kernel_spmd

    cfg = VARIANTS[variant]
    key = f"nc_{variant}"
    if key not in _CACHE:
        _CACHE[key] = build_nc(**cfg)
    nc = _CACHE[key]
    in_maps = _prep_inputs(**inputs, fp8_full=cfg["fp8_full"])
    res = run_bass_kernel_spmd(
        nc, in_maps, core_ids=list(range(B)), trace=trace
    )
    _CACHE["last_exec_time_ns"] = res.exec_time_ns
    _CACHE["last_results"] = res
    out = np.stack([np.asarray(r["out"]) for r in res.results])
    return out.reshape(B, C, 64, 64).astype(np.float32)


def _sane(out, x):
    """Cheap output plausibility: out = x + small attention path."""
    if not np.isfinite(out).all():
        return False
    d = out - x.reshape(out.shape)
    rms = float(np.sqrt((d.astype(np.float64) ** 2).mean()))
    return 1e-6 < rms < 0.5


DEFAULT_ORDER = ["fp8", "bf16"]


def kernel(**inputs):
    import os

    x = np.asarray(inputs["x"], np.float32)
    mode = os.environ.get("ATTN_KERNEL_VARIANT", "auto")
    order = DEFAULT_ORDER if mode == "auto" else [mode]
    out = None
    for variant in order:
        try:
            out = _run_variant(variant, inputs)
        except Exception:
            if variant is order[-1]:
                raise
            continue
        if _sane(out, x) or variant is order[-1]:
            return out
    return out


def last_exec_time_ns():
    return _CACHE.get("last_exec_time_ns")


def run_traced(variant, **inputs):
    """Test helper: run one variant with NTFF tracing, return (out, results)."""
    out = _run_variant(variant, inputs, trace=True)
    return out, _CACHE["last_results"]
